# revision 15
# baseline (speedup 1.0000x reference)
"""Trainium2 Bass kernel for additive-attention nn.Module.

Math: reference computes
    scores[b,i,j] = x[b,i,:]@W[0,:3] + key[b,j,:]@W[0,3:] + b0
    attn = softmax(scores, axis=j) ; out = attn @ value

softmax over j is shift-invariant, so the x- and bias-terms (constant in j)
cancel exactly: attn[b,i,j] = softmax_j(key[b,j,:]@W[0,3:]) independent of i.
Hence out[b,i,:] = sum_j p[b,j] * value[b,j,:]  (identical for every i).

The device computes only the unique rows out_row[b,:] = (sum_j e[b,j] *
value[b,j,:]) / s[b]; replicating them across the S1 axis is pure output
unsharding and happens on the host. This halves device HBM traffic vs
writing the full (B, S1, DV) tensor: per core it reads 8 MB of value and
writes 8 KB.

Kernel (data-parallel over batch, 8 batches/core on 8 cores):
  value SBUF layout: partition q holds rows j=8q..8q+7 (8 KB contiguous
  DMA per partition). key is pre-transposed on the host so the logits are
  computed directly in the matching layout eT[q, jj*8+b] = e[b, 8q+jj]:
  1. sk = key_r . w_k         (3 DVE fused mul-adds on [128, 64])
  2. eT = exp(sk)             (ACT, [128, 64])
  3. s via ones-matmul        (PE: [128,1]^T @ [128,64] -> [1,64]),
     tree-add over jj -> [1,8], reciprocal -> r_row (off critical path)
  4. out_row[b] = sum_jj eT[:, jj*8+b]^T @ v[b][:, jj*256:...]
     8 accumulating float32r matmuls [128,1]x[128,256] per batch (PE)
  5. normalize while copying PSUM->SBUF: o_sb[0, b*256:] = acc * r[b]
     (DVE/ACT alternating), single 8 KB DMA out at the end.

Value arrives as 16 pipelined 512 KB DMAs on the sync HWDGE ring (FIFO,
in batch order) so per-batch matmuls overlap the remaining stream; control
tensors ride the scalar HWDGE ring concurrently.
"""

import numpy as np
from contextlib import ExitStack

import concourse.bass as bass
import concourse.bacc as bacc
import concourse.mybir as mybir
from concourse import tile
from concourse.bass_utils import run_bass_kernel_spmd

B, S1, S2, DV = 64, 1024, 1024, 256
NCORES = 8
BPC = B // NCORES            # batches per core
NJ = S2 // 128               # j-slots per partition (8)
F32 = mybir.dt.float32
F32R = mybir.dt.float32r

_compiled = {}


def _build_nc():
    nc = bacc.Bacc("TRN2", target_bir_lowering=False, debug=False,
                   num_devices=NCORES)

    # ctrl[q, 0:192] = key_r (key_r[q, (jj*8+b)*3+f] = key[b, 8q+jj, f],
    # host pre-transposed); ctrl[q, 192:195] = w_k broadcast per partition
    ctrl_d = nc.dram_tensor("ctrl", [128, NJ * BPC * 3 + 3], F32,
                            kind="ExternalInput")
    val_d = nc.dram_tensor("value", [BPC, S2, DV], F32R, kind="ExternalInput")
    out_d = nc.dram_tensor("out", [1, BPC * DV], F32, kind="ExternalOutput")

    with tile.TileContext(nc) as tc, ExitStack() as ctx:
        const = ctx.enter_context(tc.tile_pool(name="const", bufs=1))
        sm = ctx.enter_context(tc.tile_pool(name="sm", bufs=1))
        vpool = ctx.enter_context(tc.tile_pool(name="v", bufs=BPC))
        ps_misc = ctx.enter_context(
            tc.tile_pool(name="ps_misc", bufs=1, space=bass.MemorySpace.PSUM))
        ps_acc = ctx.enter_context(
            tc.tile_pool(name="ps_acc", bufs=4, space=bass.MemorySpace.PSUM))

        # one control DMA, first on the ACT HWDGE ring
        ctrl_sb = const.tile([128, NJ * BPC * 3 + 3], F32)
        nc.scalar.dma_start(ctrl_sb[:], ctrl_d[:])
        kr_sb = ctrl_sb[:, 0:NJ * BPC * 3]
        wk_sb = ctrl_sb[:, NJ * BPC * 3:NJ * BPC * 3 + 3]
        ones_f = const.tile([128, 1], F32)
        nc.vector.memset(ones_f[:], 1.0)
        ones_sb = const.tile([128, 1], F32R)
        nc.vector.tensor_copy(ones_sb[:], ones_f[:])

        # value stream: one DMA per batch, alternating between the SP HWDGE
        # ring (even) and the GpSimd SWDGE ring (odd) so descriptor
        # generation runs in parallel and batch data arrives pipelined in
        # batch order. The ACT ring carries only the ctrl DMA + exp +
        # copies: value triggers stall on ring capacity, and anything
        # queued behind them (program order per engine) would inherit that
        # stall — exp gates every matmul, so it must not share a queue
        # with the stream.
        # partition q holds rows 8q..8q+7 of value[b] -> 8 KB contiguous
        # per partition. The first batch on each ring leads with a small
        # piece (engines start draining sooner: the ring TAIL is bumped per
        # DMA); the last batch on each ring trails with small pieces so the
        # post-arrival matmul work is ~1 matmul, not 8.
        v_tiles = []
        W_ = NJ * DV
        for b in range(BPC):
            v_sb = vpool.tile([128, W_], F32R, tag="v_sb")
            v_src = val_d.ap()[b].rearrange("(q jj) d -> q (jj d)", q=128)
            eng = nc.sync if b % 2 == 0 else nc.gpsimd
            if b < 2:
                cuts = (0, W_ // 8, W_)
            elif b >= BPC - 2:
                cuts = (0, W_ // 2, 3 * W_ // 4, W_)
            else:
                cuts = (0, W_)
            for lo, hi in zip(cuts[:-1], cuts[1:]):
                eng.dma_start(v_sb[:, lo:hi], v_src[:, lo:hi])
            v_tiles.append(v_sb)

        # logits in transposed layout: sk[q, jj*8+b] = key_r . w_k
        k3 = kr_sb.rearrange("q (c f) -> q c f", f=3)
        sk0 = sm.tile([128, NJ * BPC], F32)
        sk1 = sm.tile([128, NJ * BPC], F32)
        eT = sm.tile([128, NJ * BPC], F32R)
        nc.vector.tensor_scalar_mul(sk0[:], k3[:, :, 0], wk_sb[:, 0:1])
        nc.vector.scalar_tensor_tensor(
            sk1[:], k3[:, :, 1], wk_sb[:, 1:2], sk0[:],
            op0=mybir.AluOpType.mult, op1=mybir.AluOpType.add)
        nc.vector.scalar_tensor_tensor(
            sk0[:], k3[:, :, 2], wk_sb[:, 2:3], sk1[:],
            op0=mybir.AluOpType.mult, op1=mybir.AluOpType.add)

        # eT = exp(sk)  (unnormalized softmax numerator, transposed layout)
        nc.scalar.activation(eT[:], sk0[:], mybir.ActivationFunctionType.Exp,
                             bias=0.0, scale=1.0)

        # softmax denominators: column-sums via ones-matmul, then reduce the
        # NJ j-slots per batch and invert. Off the matmul critical path.
        s_ps = ps_misc.tile([1, NJ * BPC], F32)
        nc.tensor.matmul(s_ps[:], ones_sb[:], eT[:], start=True, stop=True)
        s_sb = sm.tile([1, NJ * BPC], F32)
        nc.vector.tensor_copy(s_sb[:], s_ps[:])
        s_v = s_sb[:].rearrange("p (jj b) -> p jj b", b=BPC)
        t32 = sm.tile([1, 4 * BPC], F32)
        t32v = t32[:].rearrange("p (jj b) -> p jj b", b=BPC)
        nc.vector.tensor_add(t32v[:, 0:4, :], s_v[:, 0:4, :], s_v[:, 4:8, :])
        nc.vector.tensor_add(t32v[:, 0:2, :], t32v[:, 0:2, :], t32v[:, 2:4, :])
        nc.vector.tensor_add(t32v[:, 0:1, :], t32v[:, 0:1, :], t32v[:, 1:2, :])
        r_row = sm.tile([1, BPC], F32)
        nc.vector.reciprocal(r_row[:], t32[:, 0:BPC])

        # per-batch weighted sums on the PE: 8 accumulating float32r
        # matmuls [128,1] x [128,256] -> [1,256] per batch
        o_sb = sm.tile([1, BPC * DV], F32)
        for b in range(BPC):
            v_sb = v_tiles[b]
            acc = ps_acc.tile([1, DV], F32, tag="acc")
            for jj in range(NJ):
                col = jj * BPC + b
                nc.tensor.matmul(
                    acc[:],
                    eT[:, col:col + 1],
                    v_sb[:, jj * DV:(jj + 1) * DV],
                    start=(jj == 0), stop=(jj == NJ - 1))
            # normalize while evacuating PSUM; alternate DVE/ACT
            dst = o_sb[:, b * DV:(b + 1) * DV]
            if b % 2 == 0:
                nc.vector.tensor_scalar_mul(dst, acc[:], r_row[:, b:b + 1])
            else:
                nc.scalar.mul(dst, acc[:], r_row[:, b:b + 1])

        nc.sync.dma_start(out_d[:], o_sb[:])

    nc.compile()
    return nc


def _get_nc():
    if "nc" not in _compiled:
        _compiled["nc"] = _build_nc()
    return _compiled["nc"]


def _make_in_maps(key, value, W):
    key = np.ascontiguousarray(np.asarray(key, dtype=np.float32))
    value = np.asarray(value, dtype=np.float32)
    W = np.asarray(W, dtype=np.float32)
    wk128 = np.tile(W[0, 3:].reshape(1, 3), (128, 1))
    in_maps = []
    for c in range(NCORES):
        lo, hi = c * BPC, (c + 1) * BPC
        # key_r[q, jj, b, f] = key[b, 8q+jj, f]
        kc = key[lo:hi].reshape(BPC, 128, NJ, 3)
        keyr = kc.transpose(1, 2, 0, 3).reshape(128, NJ * BPC * 3)
        ctrl = np.ascontiguousarray(
            np.concatenate([keyr, wk128], axis=1))
        in_maps.append({
            "ctrl": ctrl,
            "value": np.ascontiguousarray(value[lo:hi]),
        })
    return in_maps


def kernel(x, key, value, W, b):
    nc = _get_nc()
    in_maps = _make_in_maps(key, value, W)
    res = run_bass_kernel_spmd(nc, in_maps, core_ids=list(range(NCORES)))
    rows = np.concatenate(
        [r["out"].reshape(BPC, DV) for r in res.results], axis=0)
    return np.ascontiguousarray(
        np.broadcast_to(rows[:, None, :], (B, S1, DV)))


def kernel_traced(x, key, value, W, b, **spmd_kwargs):
    """Like kernel() but returns (output, BassKernelResults) — for test.py."""
    nc = _get_nc()
    in_maps = _make_in_maps(key, value, W)
    res = run_bass_kernel_spmd(nc, in_maps, core_ids=list(range(NCORES)),
                               **spmd_kwargs)
    rows = np.concatenate(
        [r["out"].reshape(BPC, DV) for r in res.results], axis=0)
    out = np.ascontiguousarray(np.broadcast_to(rows[:, None, :], (B, S1, DV)))
    return out, res
